# revision 3
# baseline (speedup 1.0000x reference)
"""Trainium2 Bass kernel for ContinuousLatticeGPT (dense transformer).

Self-contained: takes full (unsharded) inputs as numpy arrays, shards
batch-parallel across 8 NeuronCores, runs a fused Bass/Tile kernel per core,
gathers the full output.

Design notes:
- Data parallel over batch: B=32 -> 4 batches/core; weights + attn bias
  replicated.
- Activations live FEATURE-MAJOR ([d, token]) on-chip so every matmul needs
  no transposes: out_fm = matmul(lhsT=W, rhs=x_fm). Exceptions: v is produced
  token-major (lhsT=h_fm slice), attention probs are built transposed
  ([k, q]) so attn@v needs no transpose either.
- Softmax: exp(bias + causal_mask) is precomputed on host (exact zeros above
  the diagonal); in-kernel softmax is exp(scores) * expb with denominators
  collected by an extra all-ones column appended to v.
- LayerNorm stats over the feature (partition) dim via ones-matmul on PE,
  which also broadcasts the stats to all partitions for free.
- fp16 matmul operands everywhere (1 cyc/row on PE like bf16, ~8x less
  rounding error); fp32 accumulation in PSUM; residual stream in fp16.
"""

import sys
import functools

import numpy as np

sys.path.insert(0, "/opt/trn_rl_repo")

import concourse.bass as bass  # noqa: E402
import concourse.tile as tile  # noqa: E402
from concourse import bacc, mybir  # noqa: E402
from concourse.bass_utils import run_bass_kernel_spmd  # noqa: E402

# model dims
B, S, D, H, NL, DFF, C, K = 32, 1024, 512, 8, 6, 2048, 128, 8
HD = D // H  # 64
EPS = 1e-5
NCORES = 8
BL = B // NCORES          # 4 batches per core
T = BL * S                # 4096 tokens per core
TCH = 512                 # token chunk
NTC = T // TCH            # 8
DC = D // 128             # 4 d-chunks
FC = DFF // 128           # 16 dff chunks
KT = S // 128             # 8 k-tiles per sequence

F32 = mybir.dt.float32
F16 = mybir.dt.float16
AF = mybir.ActivationFunctionType
ALU = mybir.AluOpType

INV_SQRT_HD = 1.0 / np.sqrt(HD)


def build_nc(nl=NL):
    nc = bacc.Bacc("TRN2", target_bir_lowering=False, debug=False,
                   num_devices=NCORES)

    # ---- DRAM I/O (per-core shapes) ----
    cs_d = nc.dram_tensor("cs", [2, T], F32, kind="ExternalInput")
    condT_d = nc.dram_tensor("condT", [C, BL], F32, kind="ExternalInput")
    expb_d = nc.dram_tensor("expbT", [H, S, S], F16, kind="ExternalInput")
    embW_d = nc.dram_tensor("embW", [2, D], F32, kind="ExternalInput")
    embB_d = nc.dram_tensor("embB", [D], F32, kind="ExternalInput")
    adaW_d = nc.dram_tensor("adaW", [NL, 2, C, 2 * D], F32, kind="ExternalInput")
    adaB_d = nc.dram_tensor("adaB", [NL, 2, 2 * D], F32, kind="ExternalInput")
    lng_d = nc.dram_tensor("lng", [NL, 2, D], F32, kind="ExternalInput")
    lnb_d = nc.dram_tensor("lnb", [NL, 2, D], F32, kind="ExternalInput")
    wq_d = nc.dram_tensor("wq", [NL, D, D], F16, kind="ExternalInput")
    wk_d = nc.dram_tensor("wk", [NL, D, D], F16, kind="ExternalInput")
    wv_d = nc.dram_tensor("wv", [NL, D, D], F16, kind="ExternalInput")
    wo_d = nc.dram_tensor("wo", [NL, D, D], F16, kind="ExternalInput")
    w1_d = nc.dram_tensor("w1", [NL, D, DFF], F16, kind="ExternalInput")
    w2_d = nc.dram_tensor("w2", [NL, DFF, D], F16, kind="ExternalInput")
    bq_d = nc.dram_tensor("bq125", [NL, D], F32, kind="ExternalInput")
    bk_d = nc.dram_tensor("bk", [NL, D], F32, kind="ExternalInput")
    bo_d = nc.dram_tensor("boeff", [NL, D], F32, kind="ExternalInput")
    b1_d = nc.dram_tensor("b1", [NL, DFF], F32, kind="ExternalInput")
    b2_d = nc.dram_tensor("b2", [NL, D], F32, kind="ExternalInput")
    fng_d = nc.dram_tensor("fng", [D], F32, kind="ExternalInput")
    fnb_d = nc.dram_tensor("fnb", [D], F32, kind="ExternalInput")
    hw_d = nc.dram_tensor("headW", [D, 3 * K], F16, kind="ExternalInput")
    out_d = nc.dram_tensor("out", [T, 3 * K], F32, kind="ExternalOutput")

    import contextlib
    with tile.TileContext(nc) as tc, contextlib.ExitStack() as ctx:
        # ---- pools ----
        pers = ctx.enter_context(tc.tile_pool(name="pers", bufs=1))
        ps = ctx.enter_context(tc.tile_pool(name="ps", bufs=4, space="PSUM"))
        dram = ctx.enter_context(tc.tile_pool(name="dram", bufs=1, space="DRAM"))
        wts = ctx.enter_context(tc.tile_pool(name="wts", bufs=1))
        big = ctx.enter_context(tc.tile_pool(name="big", bufs=2))
        stat = ctx.enter_context(tc.tile_pool(name="stat", bufs=2))
        work = ctx.enter_context(tc.tile_pool(name="work", bufs=2))
        stg = ctx.enter_context(tc.tile_pool(name="stg", bufs=3))
        cols = ctx.enter_context(tc.tile_pool(name="cols", bufs=2))

        # ---- DRAM scratch ----
        qd = dram.tile([D, T], F16, tag="qd")
        kd = dram.tile([D, T], F16, tag="kd")
        vd = dram.tile([BL, H, KT, 128, HD + 1], F16, tag="vd")
        aod = dram.tile([D, T], F16, tag="aod")

        # ---- persistent SBUF ----
        x = [pers.tile([128, T], F16, tag=f"x{c}", name=f"x{c}") for c in range(DC)]
        ones16 = pers.tile([128, 128], F16, tag="ones16", name="ones16")
        nc.vector.memset(ones16, 1.0)
        condT = pers.tile([C, BL], F32, tag="condT", name="condT")
        nc.sync.dma_start(out=condT, in_=condT_d[:, :])
        eps_col = pers.tile([128, 1], F32, tag="eps", name="eps_col")
        nc.vector.memset(eps_col, EPS)

        # ================= embedding =================
        embW = cols.tile([2, D], F32, tag="embW", name="embW")
        nc.sync.dma_start(out=embW, in_=embW_d[:, :])
        embB = cols.tile([128, DC], F32, tag="embB", name="embB")
        nc.sync.dma_start(out=embB, in_=embB_d[:].rearrange("(c p) -> p c", p=128))
        for t in range(NTC):
            csb = stg.tile([2, TCH], F32, tag="csb", name="csb")
            nc.sync.dma_start(out=csb, in_=cs_d[:, t * TCH:(t + 1) * TCH])
            for m in range(DC):
                pe = ps.tile([128, TCH], F32, tag="ps", name="pe")
                nc.tensor.matmul(pe, embW[:, m * 128:(m + 1) * 128], csb,
                                 start=True, stop=True)
                nc.scalar.activation(x[m][:, t * TCH:(t + 1) * TCH], pe,
                                     AF.Identity, bias=embB[:, m:m + 1])

        # helper: AdaLN / LN over a token chunk -> h tiles (fp16)
        def emit_ln(t, a_col_fn, s_col_fn, tg):
            """LayerNorm over feature dim for token chunk t, then per-chunk
            affine h = xhat * A + S with per-partition A/S columns."""
            tsl = slice(t * TCH, (t + 1) * TCH)
            psum = ps.tile([128, TCH], F32, tag="ps", name="psum")
            psq = ps.tile([128, TCH], F32, tag="ps", name="psq")
            for c in range(DC):
                sq = stat.tile([128, TCH], F16, tag="sq", name="sq")
                nc.scalar.square(sq, x[c][:, tsl])
                nc.tensor.matmul(psum, ones16, x[c][:, tsl],
                                 start=(c == 0), stop=(c == DC - 1))
                nc.tensor.matmul(psq, ones16, sq,
                                 start=(c == 0), stop=(c == DC - 1))
            m = stat.tile([128, TCH], F32, tag="m", name="m")
            nc.scalar.mul(m, psum, 1.0 / D)
            v2 = stat.tile([128, TCH], F32, tag="v2", name="v2")
            nc.scalar.mul(v2, psq, 1.0 / D)
            msq = stat.tile([128, TCH], F32, tag="msq", name="msq")
            nc.vector.tensor_mul(msq, m, m)
            nc.vector.tensor_sub(v2, v2, msq)
            # rstd = 1/sqrt(var + eps)
            nc.scalar.activation(v2, v2, AF.Sqrt, bias=eps_col)
            nc.vector.reciprocal(v2, v2)
            hs = []
            for c in range(DC):
                xc = stat.tile([128, TCH], F32, tag="xc", name="xc")
                nc.vector.tensor_sub(xc, x[c][:, tsl], m)
                nc.vector.tensor_mul(xc, xc, v2)
                hc = work.tile([128, TCH], F16, tag=f"h{c}_{tg}", name=f"h{c}")
                nc.vector.tensor_scalar(hc, xc, a_col_fn(c), s_col_fn(c),
                                        op0=ALU.mult, op1=ALU.add)
                hs.append(hc)
            return hs

        # ================= layers =================
        for li in range(nl):
            # --- per-layer weight / bias loads ---
            wq = wts.tile([128, DC, D], F16, tag="wq", name="wq")
            nc.sync.dma_start(out=wq, in_=wq_d[li].rearrange("(c p) o -> p c o", p=128))
            wk = wts.tile([128, DC, D], F16, tag="wk", name="wk")
            nc.sync.dma_start(out=wk, in_=wk_d[li].rearrange("(c p) o -> p c o", p=128))
            wv = wts.tile([128, DC, D], F16, tag="wv", name="wv")
            nc.sync.dma_start(out=wv, in_=wv_d[li].rearrange("(c p) o -> p c o", p=128))
            wo = wts.tile([128, DC, D], F16, tag="wo", name="wo")
            nc.sync.dma_start(out=wo, in_=wo_d[li].rearrange("(c p) o -> p c o", p=128))
            w2 = wts.tile([128, FC, D], F16, tag="w2", name="w2")
            nc.sync.dma_start(out=w2, in_=w2_d[li].rearrange("(f p) o -> p f o", p=128))

            adaW = wts.tile([128, 2, 8, 128], F32, tag="adaW", name="adaW")
            nc.sync.dma_start(out=adaW,
                              in_=adaW_d[li].rearrange("a c (m p) -> c a m p", p=128))
            adab = cols.tile([128, 2, 8], F32, tag="adab", name="adab")
            nc.sync.dma_start(out=adab,
                              in_=adaB_d[li].rearrange("a (m p) -> p a m", p=128))
            lng = cols.tile([128, 2, DC], F32, tag="lng", name="lng")
            nc.sync.dma_start(out=lng,
                              in_=lng_d[li].rearrange("a (c p) -> p a c", p=128))
            lnb = cols.tile([128, 2, DC], F32, tag="lnb", name="lnb")
            nc.sync.dma_start(out=lnb,
                              in_=lnb_d[li].rearrange("a (c p) -> p a c", p=128))
            bqc = cols.tile([128, DC], F32, tag="bqc", name="bqc")
            nc.sync.dma_start(out=bqc, in_=bq_d[li].rearrange("(c p) -> p c", p=128))
            bkc = cols.tile([128, DC], F32, tag="bkc", name="bkc")
            nc.sync.dma_start(out=bkc, in_=bk_d[li].rearrange("(c p) -> p c", p=128))
            boc = cols.tile([128, DC], F32, tag="boc", name="boc")
            nc.sync.dma_start(out=boc, in_=bo_d[li].rearrange("(c p) -> p c", p=128))
            b1c = cols.tile([128, FC], F32, tag="b1c", name="b1c")
            nc.sync.dma_start(out=b1c, in_=b1_d[li].rearrange("(f p) -> p f", p=128))
            b2c = cols.tile([128, DC], F32, tag="b2c", name="b2c")
            nc.sync.dma_start(out=b2c, in_=b2_d[li].rearrange("(c p) -> p c", p=128))

            # --- AdaLN scale/shift (per batch) for both LNs ---
            ssb = cols.tile([128, 2, 8, BL], F32, tag="ssb", name="ssb")
            for a in range(2):
                for mch in range(8):
                    pss = ps.tile([128, BL], F32, tag="ps", name="pss")
                    nc.tensor.matmul(pss, adaW[:, a, mch, :], condT,
                                     start=True, stop=True)
                    nc.scalar.activation(ssb[:, a, mch, :], pss, AF.Identity,
                                         bias=adab[:, a, mch:mch + 1])
            # A = (1 + scale) * g ; S = (1 + scale) * b + shift
            Asb = cols.tile([128, 2, DC, BL], F32, tag="Asb", name="Asb")
            Ssb = cols.tile([128, 2, DC, BL], F32, tag="Ssb", name="Ssb")
            for a in range(2):
                for c in range(DC):
                    t1 = cols.tile([128, BL], F32, tag="t1", name="t1")
                    nc.vector.tensor_scalar_add(t1, ssb[:, a, c, :], 1.0)
                    nc.vector.tensor_scalar(Asb[:, a, c, :], t1,
                                            lng[:, a, c:c + 1], None, op0=ALU.mult)
                    t2 = cols.tile([128, BL], F32, tag="t2", name="t2")
                    nc.vector.tensor_scalar(t2, t1, lnb[:, a, c:c + 1], None,
                                            op0=ALU.mult)
                    nc.vector.tensor_add(Ssb[:, a, c, :], t2, ssb[:, a, DC + c, :])

            # ---------- Phase A: AdaLN1 + QKV ----------
            for t in range(NTC):
                b = t // (NTC // BL)
                tsl = slice(t * TCH, (t + 1) * TCH)
                hs = emit_ln(t,
                             lambda c: Asb[:, 0, c, b:b + 1],
                             lambda c: Ssb[:, 0, c, b:b + 1], "a")
                # q, k (feature-major out)
                for wmat, bias_col, scl, dst in ((wq, bqc, INV_SQRT_HD, qd),
                                                 (wk, bkc, 1.0, kd)):
                    for m in range(DC):
                        pq = ps.tile([128, TCH], F32, tag="ps", name="pq")
                        for c in range(DC):
                            nc.tensor.matmul(pq, wmat[:, c, m * 128:(m + 1) * 128],
                                             hs[c], start=(c == 0),
                                             stop=(c == DC - 1))
                        qs = stg.tile([128, TCH], F16, tag="qs", name="qs")
                        nc.scalar.activation(qs, pq, AF.Identity,
                                             bias=bias_col[:, m:m + 1], scale=scl)
                        nc.sync.dma_start(
                            out=dst[m * 128:(m + 1) * 128, tsl], in_=qs)
                # v (token-major out, with ones column per head)
                for kt in range(TCH // 128):
                    gkt = (t % (NTC // BL)) * (TCH // 128) + kt
                    pv = ps.tile([128, D], F32, tag="ps", name="pv")
                    for c in range(DC):
                        nc.tensor.matmul(
                            pv, hs[c][:, kt * 128:(kt + 1) * 128], wv[:, c, :],
                            start=(c == 0), stop=(c == DC - 1))
                    vs = stg.tile([128, H * (HD + 1)], F16, tag="vs", name="vs")
                    vs3 = vs.rearrange("p (h c) -> p h c", c=HD + 1)
                    nc.scalar.copy(vs3[:, :, 0:HD],
                                   pv.rearrange("p (h c) -> p h c", c=HD))
                    nc.vector.memset(vs3[:, :, HD:HD + 1], 1.0)
                    nc.sync.dma_start(
                        out=vd[b].rearrange("h k p c -> p h k c")[:, :, gkt, :],
                        in_=vs3)

            # ---------- Phase B: attention ----------
            for hp in range(H // 2):
                qp = big.tile([128, T], F16, tag="qp", bufs=1, name="qp")
                kp = big.tile([128, T], F16, tag="kp", bufs=1, name="kp")
                for hh in range(2):
                    h = 2 * hp + hh
                    nc.sync.dma_start(out=qp[hh * 64:(hh + 1) * 64, :],
                                      in_=qd[h * HD:(h + 1) * HD, :])
                    nc.sync.dma_start(out=kp[hh * 64:(hh + 1) * 64, :],
                                      in_=kd[h * HD:(h + 1) * HD, :])
                for hh in range(2):
                    h = 2 * hp + hh
                    base = 64 * hh
                    ebs = []
                    for half in range(2):
                        eb = big.tile([128, 4, S], F16, tag="big", name="eb")
                        nc.sync.dma_start(
                            out=eb,
                            in_=expb_d[h, half * 512:(half + 1) * 512, :]
                            .rearrange("(kc p) q -> p kc q", p=128))
                        ebs.append(eb)
                    for b in range(BL):
                        bsl = slice(b * S, (b + 1) * S)
                        vbh = stg.tile([128, KT, HD + 1], F16, tag="vbh",
                                       name="vbh")
                        nc.sync.dma_start(
                            out=vbh, in_=vd[b, h].rearrange("k p c -> p k c"))
                        pao = ps.tile([HD + 1, S], F32, tag="ps", name="pao")
                        for kc in range(KT):
                            psc = ps.tile([128, S], F32, tag="ps", name="psc")
                            for q2 in range(2):
                                nc.tensor.matmul(
                                    psc[:, q2 * 512:(q2 + 1) * 512],
                                    kp[base:base + 64,
                                       b * S + kc * 128:b * S + (kc + 1) * 128],
                                    qp[base:base + 64,
                                       b * S + q2 * 512:b * S + (q2 + 1) * 512],
                                    start=True, stop=True)
                            et = stg.tile([128, S], F16, tag="et", name="et")
                            nc.scalar.activation(et, psc, AF.Exp)
                            nc.vector.tensor_mul(et, et, ebs[kc // 4][:, kc % 4, :])
                            for q2 in range(2):
                                nc.tensor.matmul(
                                    pao[:, q2 * 512:(q2 + 1) * 512],
                                    vbh[:, kc, :],
                                    et[:, q2 * 512:(q2 + 1) * 512],
                                    start=(kc == 0), stop=(kc == KT - 1))
                        # normalize by denominator (row HD of pao)
                        dnr = work.tile([1, S], F32, tag="dnr", name="dnr")
                        nc.scalar.copy(dnr, pao[HD:HD + 1, :])
                        nc.vector.reciprocal(dnr, dnr)
                        bc = work.tile([HD, S], F32, tag="bc", name="bc")
                        nc.gpsimd.partition_broadcast(bc, dnr)
                        aos = stg.tile([HD, S], F16, tag="aos", name="aos")
                        nc.vector.tensor_mul(aos, pao[0:HD, :], bc)
                        nc.sync.dma_start(out=aod[h * HD:(h + 1) * HD, bsl],
                                          in_=aos)

            # ---------- O-projection + residual ----------
            for t in range(NTC):
                tsl = slice(t * TCH, (t + 1) * TCH)
                aor = []
                for c in range(DC):
                    ar = stg.tile([128, TCH], F16, tag="aor", bufs=5, name="ar")
                    nc.sync.dma_start(out=ar,
                                      in_=aod[c * 128:(c + 1) * 128, tsl])
                    aor.append(ar)
                for m in range(DC):
                    po = ps.tile([128, TCH], F32, tag="ps", name="po")
                    for c in range(DC):
                        nc.tensor.matmul(po, wo[:, c, m * 128:(m + 1) * 128],
                                         aor[c], start=(c == 0),
                                         stop=(c == DC - 1))
                    nc.vector.scalar_tensor_tensor(
                        x[m][:, tsl], po, boc[:, m:m + 1], x[m][:, tsl],
                        op0=ALU.add, op1=ALU.add)

            # ---------- Phase C: AdaLN2 + FFN ----------
            w1t = []
            for wh in range(2):
                w1h = big.tile([128, 2, DFF], F16, tag="big", name="w1h")
                nc.sync.dma_start(
                    out=w1h,
                    in_=w1_d[li, wh * 256:(wh + 1) * 256, :]
                    .rearrange("(a p) f -> p a f", p=128))
                w1t.append(w1h)
            for t in range(NTC):
                b = t // (NTC // BL)
                tsl = slice(t * TCH, (t + 1) * TCH)
                hs = emit_ln(t,
                             lambda c: Asb[:, 1, c, b:b + 1],
                             lambda c: Ssb[:, 1, c, b:b + 1], "a")
                gel = big.tile([128, FC, TCH], F16, tag="gelu", bufs=1,
                               name="gel")
                for f in range(FC):
                    pg = ps.tile([128, TCH], F32, tag="ps", name="pg")
                    for c in range(DC):
                        nc.tensor.matmul(
                            pg, w1t[c // 2][:, c % 2, f * 128:(f + 1) * 128],
                            hs[c], start=(c == 0), stop=(c == DC - 1))
                    nc.scalar.activation(gel[:, f, :], pg, AF.Gelu,
                                         bias=b1c[:, f:f + 1])
                for m in range(DC):
                    po2 = ps.tile([128, TCH], F32, tag="ps", name="po2")
                    for f in range(FC):
                        nc.tensor.matmul(po2, w2[:, f, m * 128:(m + 1) * 128],
                                         gel[:, f, :], start=(f == 0),
                                         stop=(f == FC - 1))
                    nc.vector.scalar_tensor_tensor(
                        x[m][:, tsl], po2, b2c[:, m:m + 1], x[m][:, tsl],
                        op0=ALU.add, op1=ALU.add)

        # ================= final LN + head =================
        fng = cols.tile([128, DC], F32, tag="fng", name="fng")
        nc.sync.dma_start(out=fng, in_=fng_d[:].rearrange("(c p) -> p c", p=128))
        fnb = cols.tile([128, DC], F32, tag="fnb", name="fnb")
        nc.sync.dma_start(out=fnb, in_=fnb_d[:].rearrange("(c p) -> p c", p=128))
        hw = cols.tile([128, DC, 3 * K], F16, tag="hw", name="hw")
        nc.sync.dma_start(out=hw, in_=hw_d[:].rearrange("(c p) o -> p c o", p=128))
        for t in range(NTC):
            hs = emit_ln(t,
                         lambda c: fng[:, c:c + 1],
                         lambda c: fnb[:, c:c + 1], "a")
            for kt in range(TCH // 128):
                ph = ps.tile([128, 3 * K], F32, tag="ps", name="ph")
                for c in range(DC):
                    nc.tensor.matmul(ph, hs[c][:, kt * 128:(kt + 1) * 128],
                                     hw[:, c, :], start=(c == 0),
                                     stop=(c == DC - 1))
                osb = stg.tile([128, 3 * K], F32, tag="osb", name="osb")
                nc.scalar.copy(osb, ph)
                nc.sync.dma_start(
                    out=out_d[t * TCH + kt * 128:t * TCH + (kt + 1) * 128, :],
                    in_=osb)

    nc.compile()
    return nc


@functools.lru_cache(maxsize=2)
def _get_nc(nl=NL):
    return build_nc(nl)


def _prep_shared(theta, cond, attn_bias, embed_W, embed_b, ln1_g, ln1_b,
                 ada1_W, ada1_b, Wq, bq, Wk, bk, Wv, bv, Wo, bo, ln2_g, ln2_b,
                 ada2_W, ada2_b, W1, b1, W2, b2, fn_g, fn_b, head_W, head_b):
    """Host-side preprocessing shared across cores."""
    f16 = np.float16
    f32 = np.float32
    mask = np.triu(np.ones((S, S), bool), k=1)
    biasm = attn_bias.astype(f32) + np.where(mask, -np.inf, 0.0).astype(f32)
    expbT = np.exp(biasm).transpose(0, 2, 1).astype(f16)  # [H, k, q]
    shared = dict(
        expbT=np.ascontiguousarray(expbT),
        embW=embed_W.astype(f32),
        embB=embed_b.astype(f32),
        adaW=np.ascontiguousarray(np.stack([ada1_W, ada2_W], 1).astype(f32)),
        adaB=np.ascontiguousarray(np.stack([ada1_b, ada2_b], 1).astype(f32)),
        lng=np.ascontiguousarray(np.stack([ln1_g, ln2_g], 1).astype(f32)),
        lnb=np.ascontiguousarray(np.stack([ln1_b, ln2_b], 1).astype(f32)),
        wq=Wq.astype(f16), wk=Wk.astype(f16), wv=Wv.astype(f16),
        wo=Wo.astype(f16),
        w1=W1.astype(f16), w2=W2.astype(f16),
        bq125=(bq * INV_SQRT_HD).astype(f32),
        bk=bk.astype(f32),
        boeff=(bo + np.einsum("ld,ldo->lo", bv.astype(np.float64),
                              Wo.astype(np.float64)).astype(f32)).astype(f32),
        b1=b1.astype(f32), b2=b2.astype(f32),
        fng=fn_g.astype(f32), fnb=fn_b.astype(f32),
        headW=head_W.astype(f16),
    )
    return shared


def kernel(**inputs):
    theta = np.asarray(inputs["theta"], np.float32)
    cond = np.asarray(inputs["cond"], np.float32)
    head_b = np.asarray(inputs["head_b"], np.float32)

    shared = _prep_shared(**{k: np.asarray(v) for k, v in inputs.items()})

    in_maps = []
    for c in range(NCORES):
        th = theta[c * BL:(c + 1) * BL].reshape(-1)  # [T]
        cs = np.stack([np.cos(th), np.sin(th)]).astype(np.float32)  # [2, T]
        condT = np.ascontiguousarray(cond[c * BL:(c + 1) * BL].T.astype(np.float32))
        m = dict(shared)
        m["cs"] = cs
        m["condT"] = condT
        in_maps.append(m)

    nc = _get_nc()
    res = run_bass_kernel_spmd(nc, in_maps, list(range(NCORES)))
    outs = [np.asarray(res.results[c]["out"], np.float32) for c in range(NCORES)]
    full = np.concatenate(outs, axis=0).reshape(B, S, 3 * K)
    full = full + head_b[None, None, :]
    return full.astype(np.float32)


if __name__ == "__main__":
    nc = build_nc()
    ni = sum(len(bb.instructions) for bb in nc.main_func.blocks)
    print("instructions:", ni)
